# revision 4
# baseline (speedup 1.0000x reference)
"""MultiHead HGNN attention (B=2, S=4096, D=256, H=4) on 8 TRN2 NeuronCores.

Sharding: query rows are split 8 ways (512 rows/core); every core computes all
batches/heads for its query block. The score matrix is built k-major
(scores^T) so the probs@V contraction needs no transposes; G^T is produced
on-device via a bf16 cast + xbar DMA transpose. Softmax denominators ride as
an extra ones-column in the V operand; the 1/denominator scale is applied to
ctx^T through a rank-1 broadcast matmul.
"""

import contextlib
import ctypes
import sys
import types

import numpy as np

sys.path.insert(0, "/opt/trn_rl_repo")


def _install_axon_hooks():
    """The agent image's antenv lacks axon_hooks; provide it so bass_utils can
    NTFF-profile under axon. Harmless when profiling is never requested."""
    if "antenv.axon_hooks" in sys.modules:
        return
    try:
        import antenv
    except ImportError:
        return
    mod = types.ModuleType("antenv.axon_hooks")
    holder = {}
    mod.set_axon_ntff_profile_hook = lambda h: holder.__setitem__("h", h)
    mod.get_axon_ntff_profile_hook = lambda: holder.get("h")
    sys.modules["antenv.axon_hooks"] = mod
    antenv.axon_hooks = mod
    try:
        lib = ctypes.CDLL("/opt/axon/libaxon_pjrt.so")
    except OSError:
        return
    if not hasattr(lib, "axon_start_nrt_profile"):
        return
    lib.axon_start_nrt_profile.argtypes = [ctypes.POINTER(ctypes.c_int64), ctypes.c_size_t]
    lib.axon_start_nrt_profile.restype = ctypes.c_int64
    lib.axon_stop_nrt_profile.argtypes = [ctypes.c_char_p]
    lib.axon_stop_nrt_profile.restype = ctypes.c_int64

    @contextlib.contextmanager
    def _hook(output_dir, device_ids):
        import jax

        jax.devices()
        if device_ids:
            ids = (ctypes.c_int64 * len(device_ids))(*device_ids)
            rc = lib.axon_start_nrt_profile(ids, len(device_ids))
        else:
            rc = lib.axon_start_nrt_profile(None, 0)
        if rc != 0:
            raise RuntimeError(f"axon_start_nrt_profile rc={rc}")
        try:
            yield
        finally:
            n = lib.axon_stop_nrt_profile(str(output_dir).encode())
            print(f"profile: {n} file(s) written to {output_dir}")

    mod.set_axon_ntff_profile_hook(_hook)


_install_axon_hooks()

B, S, D, H, HD = 2, 4096, 256, 4, 64
NCORES = 8
QR = S // NCORES          # 512 query rows per core
KC = S // 128             # 32 key chunks of 128
SCALE = 1.0 / np.sqrt(HD)

_BUILT = {}


def build_bass():
    if "nc" in _BUILT:
        return _BUILT["nc"]

    import concourse.tile as tile
    from concourse import bacc, mybir

    f32, bf16 = mybir.dt.float32, mybir.dt.bfloat16
    af = mybir.ActivationFunctionType

    nc = bacc.Bacc("TRN2", target_bir_lowering=False, debug=False, num_devices=NCORES)

    xt_in = nc.dram_tensor("xt", [B, 2, 128, S], f32, kind="ExternalInput")
    xqt_in = nc.dram_tensor("xqt", [B, 2, 128, QR], f32, kind="ExternalInput")
    g_in = nc.dram_tensor("g", [QR, S], f32, kind="ExternalInput")
    wq_in = nc.dram_tensor("wq", [2, 128, 256], f32, kind="ExternalInput")
    wk_in = nc.dram_tensor("wk", [2, 128, 256], f32, kind="ExternalInput")
    wv_in = nc.dram_tensor("wv", [2, 128, 260], f32, kind="ExternalInput")
    sel_in = nc.dram_tensor("sel", [1, 260], f32, kind="ExternalInput")
    wo_in = nc.dram_tensor("wo", [H, 64, 256], f32, kind="ExternalInput")
    bias_in = nc.dram_tensor("bias", [1, 256], f32, kind="ExternalInput")
    out_dram = nc.dram_tensor("out", [B, QR, 256], f32, kind="ExternalOutput")

    with tile.TileContext(nc) as tc, contextlib.ExitStack() as ctx:
        cp = ctx.enter_context(tc.tile_pool(name="const", bufs=1))
        ps_sc = ctx.enter_context(tc.tile_pool(name="ps_sc", bufs=2, space="PSUM"))
        ps_ct = ctx.enter_context(tc.tile_pool(name="ps_ct", bufs=1, space="PSUM"))
        ps_aux = ctx.enter_context(tc.tile_pool(name="ps_aux", bufs=1, space="PSUM"))

        # ---- constants / weights ----
        wq_sb = cp.tile([128, 2, 256], f32, tag="wq")
        wk_sb = cp.tile([128, 2, 256], f32, tag="wk")
        wv_sb = cp.tile([128, 2, 260], f32, tag="wv")
        sel_sb = cp.tile([1, 260], f32, tag="sel")
        bias_sb = cp.tile([1, 256], f32, tag="bias")
        ones_sb = cp.tile([1, 128], f32, tag="ones")
        for ic in range(2):
            nc.sync.dma_start(wq_sb[:, ic, :], wq_in[ic])
            nc.sync.dma_start(wk_sb[:, ic, :], wk_in[ic])
            nc.sync.dma_start(wv_sb[:, ic, :], wv_in[ic])
        nc.sync.dma_start(sel_sb[:], sel_in[:])
        nc.sync.dma_start(bias_sb[:], bias_in[:])
        nc.gpsimd.memset(ones_sb[:], 1.0)
        wo_sb = []
        for h in range(H):
            t = cp.tile([64, 256], f32, tag=f"wo{h}", name=f"wo{h}")
            nc.sync.dma_start(t[:], wo_in[h])
            wo_sb.append(t)

        # ---- x^T (host-pretransposed) ----
        xt_sb = [[cp.tile([128, S], f32, tag=f"xt{b}{ic}", name=f"xt{b}{ic}") for ic in range(2)] for b in range(B)]
        for b in range(B):
            for ic in range(2):
                nc.sync.dma_start(xt_sb[b][ic][:], xt_in[b, ic])
        # ---- all-(b,hp) qT upfront (frees the xqt staging before main pools) ----
        gt_sb = cp.tile([128, KC, QR], bf16, tag="gt")
        qts = [[cp.tile([128, QR], f32, tag=f"qt{b}{hp}", name=f"qt{b}{hp}") for hp in range(2)] for b in range(B)]
        with tc.tile_pool(name="xqp", bufs=1) as xqp:
            xqt_sb = xqp.tile([128, B, 2, QR], f32, tag="xqt")
            for b in range(B):
                for ic in range(2):
                    nc.sync.dma_start(xqt_sb[:, b, ic, :], xqt_in[b, ic])
            for b in range(B):
                for hp in range(2):
                    aux = ps_aux.tile([128, 2, 512], f32, tag="aux")
                    for ic in range(2):
                        nc.tensor.matmul(
                            aux[:, 0, :QR], wq_sb[:, ic, hp * 128:(hp + 1) * 128],
                            xqt_sb[:, b, ic, :], start=(ic == 0), stop=(ic == 1),
                        )
                    nc.vector.tensor_copy(qts[b][hp][:], aux[:, 0, :QR])

        # ---- G^T build: load f32 rows, cast to bf16 (gpsimd), xbar-transpose ----
        with tc.tile_pool(name="graw", bufs=2) as pg, tc.tile_pool(name="gbf", bufs=2) as pgb:
            for qc in range(QR // 128):
                graw = pg.tile([128, S], f32, tag="graw")
                nc.sync.dma_start(graw[:], g_in[qc * 128:(qc + 1) * 128, :])
                gbf = pgb.tile([128, S], bf16, tag="gbf")
                nc.gpsimd.tensor_copy(gbf[:], graw[:])
                for kc in range(KC):
                    nc.sync.dma_start_transpose(
                        gt_sb[:, kc, qc * 128:(qc + 1) * 128],
                        gbf[:, kc * 128:(kc + 1) * 128],
                    )

        ktp = ctx.enter_context(tc.tile_pool(name="ktp", bufs=2))
        vap = ctx.enter_context(tc.tile_pool(name="vap", bufs=2))
        pp = ctx.enter_context(tc.tile_pool(name="pp", bufs=3))
        rp = ctx.enter_context(tc.tile_pool(name="rp", bufs=1))
        otp = ctx.enter_context(tc.tile_pool(name="otp", bufs=2))

        ctf = [[None] * H for _ in range(B)]

        for b in range(B):
            for hp in range(2):
                # ---- lazy K/V for this (batch, head-pair) ----
                qt = qts[b][hp]
                kt = ktp.tile([128, S], f32, tag="kt")
                for sc8 in range(8):
                    aux = ps_aux.tile([128, 2, 512], f32, tag="aux")
                    for ic in range(2):
                        nc.tensor.matmul(
                            aux[:, 0, :], wk_sb[:, ic, hp * 128:(hp + 1) * 128],
                            xt_sb[b][ic][:, sc8 * 512:(sc8 + 1) * 512],
                            start=(ic == 0), stop=(ic == 1),
                        )
                    nc.scalar.copy(kt[:, sc8 * 512:(sc8 + 1) * 512], aux[:, 0, :])

                va = vap.tile([128, KC, 130], bf16, tag="va")
                for sg in range(KC // 2):
                    aux = ps_aux.tile([128, 2, 512], f32, tag="aux")
                    for j in range(2):
                        kcj = sg * 2 + j
                        for ic in range(2):
                            nc.tensor.matmul(
                                aux[:, j, 0:130],
                                xt_sb[b][ic][:, kcj * 128:(kcj + 1) * 128],
                                wv_sb[:, ic, hp * 130:(hp + 1) * 130],
                                start=(ic == 0), stop=False,
                            )
                        nc.tensor.matmul(
                            aux[:, j, 0:130], ones_sb[0:1, :],
                            sel_sb[0:1, hp * 130:(hp + 1) * 130],
                            start=False, stop=True,
                        )
                    nc.vector.tensor_copy(va[:, sg * 2:(sg + 1) * 2, :], aux[:, 0:2, 0:130])

                # ---- main loop over key chunks ----
                ct0 = ps_ct.tile([65, QR], f32, tag="ct0")
                ct1 = ps_ct.tile([65, QR], f32, tag="ct1")
                for kc in range(KC):
                    scp = ps_sc.tile([128, 2, QR], f32, tag="sc")
                    nc.tensor.matmul(
                        scp[:, 0, :], kt[0:64, kc * 128:(kc + 1) * 128], qt[0:64, :],
                        start=True, stop=True, tile_position=(0, 0),
                    )
                    nc.tensor.matmul(
                        scp[:, 1, :], kt[64:128, kc * 128:(kc + 1) * 128], qt[64:128, :],
                        start=True, stop=True, tile_position=(64, 0),
                    )
                    nc.vector.tensor_mul(
                        scp[:, :, :], scp[:, :, :],
                        gt_sb[:, kc:kc + 1, :].broadcast_to([128, 2, QR]),
                    )
                    pt = pp.tile([128, 2, QR], bf16, tag="pt")
                    nc.scalar.activation(pt[:, :, :], scp[:, :, :], af.Exp)
                    nc.tensor.matmul(
                        ct0[:, :], va[:, kc, 0:65], pt[:, 0, :],
                        start=(kc == 0), stop=(kc == KC - 1),
                    )
                    nc.tensor.matmul(
                        ct1[:, :], va[:, kc, 65:130], pt[:, 1, :],
                        start=(kc == 0), stop=(kc == KC - 1),
                    )

                # ---- normalize: ctx^T * (1/denom) ----
                rec0 = rp.tile([1, QR], f32, tag="rec0")
                rec1 = rp.tile([1, QR], f32, tag="rec1")
                nc.vector.reciprocal(rec0[:], ct0[64:65, :])
                nc.vector.reciprocal(rec1[:], ct1[64:65, :])
                bc = ps_aux.tile([128, 2, 512], f32, tag="aux")
                nc.tensor.matmul(bc[0:64, 0, :QR], ones_sb[0:1, 0:64], rec0[:], start=True, stop=True)
                nc.tensor.matmul(bc[0:64, 1, :QR], ones_sb[0:1, 0:64], rec1[:], start=True, stop=True)
                c0 = cp.tile([64, QR], f32, tag=f"ctf{b}_{2 * hp}", name=f"ctf{b}_{2 * hp}")
                c1 = cp.tile([64, QR], f32, tag=f"ctf{b}_{2 * hp + 1}", name=f"ctf{b}_{2 * hp + 1}")
                nc.scalar.copy(c0[:], ct0[0:64, :])
                nc.scalar.copy(c1[:], ct1[0:64, :])
                nc.vector.tensor_mul(c0[:], c0[:], bc[0:64, 0, :QR])
                nc.vector.tensor_mul(c1[:], c1[:], bc[0:64, 1, :QR])
                ctf[b][2 * hp] = c0
                ctf[b][2 * hp + 1] = c1

            # ---- output projection for batch b ----
            for qs in range(QR // 128):
                op = ps_aux.tile([128, 2, 512], f32, tag="aux")
                for h in range(H):
                    nc.tensor.matmul(
                        op[:, 0, 0:256], ctf[b][h][:, qs * 128:(qs + 1) * 128],
                        wo_sb[h][:], start=(h == 0), stop=False,
                    )
                nc.tensor.matmul(op[:, 0, 0:256], ones_sb[0:1, :], bias_sb[0:1, :], start=False, stop=True)
                ot = otp.tile([128, 256], f32, tag="ot")
                nc.vector.tensor_copy(ot[:], op[:, 0, 0:256])
                nc.sync.dma_start(out_dram[b, qs * 128:(qs + 1) * 128, :], ot[:])

    nc.compile()
    _BUILT["nc"] = nc
    return nc


def host_inputs(x, G, Wq, Wk, Wv, Wo, bo, b_extra):
    """Build the per-core input maps (layout prep + query-row sharding)."""
    f = np.float32
    x = np.asarray(x, f)
    G = np.asarray(G, f)
    xt = np.ascontiguousarray(x.transpose(0, 2, 1)).reshape(B, 2, 128, S)
    wq = np.ascontiguousarray(np.asarray(Wq, f).T * SCALE).reshape(2, 128, 256)
    wk = np.ascontiguousarray(np.asarray(Wk, f).T).reshape(2, 128, 256)
    wvt = np.asarray(Wv, f).T  # [256 in, 256 out]
    wv = np.zeros((2, 128, 260), f)
    for hp in range(2):
        wv[:, :, hp * 130:hp * 130 + 64] = wvt[:, hp * 128:hp * 128 + 64].reshape(2, 128, 64)
        wv[:, :, hp * 130 + 65:hp * 130 + 129] = wvt[:, hp * 128 + 64:hp * 128 + 128].reshape(2, 128, 64)
    sel = np.zeros((1, 260), f)
    sel[0, [64, 129, 194, 259]] = 1.0
    wo = np.ascontiguousarray(np.asarray(Wo, f).T).reshape(H, 64, 256)
    bias = (np.asarray(bo, f) + np.asarray(b_extra, f)).reshape(1, 256)

    shared = {"xt": xt, "wq": wq, "wk": wk, "wv": wv, "sel": sel, "wo": wo, "bias": bias}
    in_maps = []
    for c in range(NCORES):
        q0 = c * QR
        m = dict(shared)
        m["g"] = np.ascontiguousarray(G[q0:q0 + QR, :])
        m["xqt"] = np.ascontiguousarray(xt[:, :, :, q0:q0 + QR])
        in_maps.append(m)
    return in_maps


def run(in_maps, trace=False):
    from concourse.bass_utils import run_bass_kernel_spmd

    nc = build_bass()
    return run_bass_kernel_spmd(nc, in_maps, core_ids=list(range(NCORES)), trace=trace)


def kernel(x, G, Wq, Wk, Wv, Wo, bo, b_extra):
    in_maps = host_inputs(x, G, Wq, Wk, Wv, Wo, bo, b_extra)
    res = run(in_maps, trace=False)
    out = np.concatenate([res.results[c]["out"] for c in range(NCORES)], axis=1)
    return out.astype(np.float32)


# revision 10
# speedup vs baseline: 1.6999x; 1.6999x over previous
"""MultiHead HGNN attention (B=2, S=4096, D=256, H=4) on 8 TRN2 NeuronCores.

Sharding: query rows are split 8 ways (512 rows/core); every core computes all
batches/heads for its query block. The score matrix is built k-major
(scores^T) so the probs@V contraction needs no transposes; G^T is produced
on-device via a bf16 cast + xbar DMA transpose. Softmax denominators ride as
an extra ones-column in the V operand; normalization is applied to ctx^T via
a broadcast of 1/denom. Matmul operands are bf16 (fp32 streams at half rate
through the PE); accumulation stays fp32 in PSUM.
"""

import contextlib
import ctypes
import sys
import types

import numpy as np

sys.path.insert(0, "/opt/trn_rl_repo")


def _install_axon_hooks():
    """The agent image's antenv lacks axon_hooks; provide it so bass_utils can
    NTFF-profile under axon. Harmless when profiling is never requested."""
    if "antenv.axon_hooks" in sys.modules:
        return
    try:
        import antenv
    except ImportError:
        return
    mod = types.ModuleType("antenv.axon_hooks")
    holder = {}
    mod.set_axon_ntff_profile_hook = lambda h: holder.__setitem__("h", h)
    mod.get_axon_ntff_profile_hook = lambda: holder.get("h")
    sys.modules["antenv.axon_hooks"] = mod
    antenv.axon_hooks = mod
    try:
        lib = ctypes.CDLL("/opt/axon/libaxon_pjrt.so")
    except OSError:
        return
    if not hasattr(lib, "axon_start_nrt_profile"):
        return
    lib.axon_start_nrt_profile.argtypes = [ctypes.POINTER(ctypes.c_int64), ctypes.c_size_t]
    lib.axon_start_nrt_profile.restype = ctypes.c_int64
    lib.axon_stop_nrt_profile.argtypes = [ctypes.c_char_p]
    lib.axon_stop_nrt_profile.restype = ctypes.c_int64

    @contextlib.contextmanager
    def _hook(output_dir, device_ids):
        import jax

        jax.devices()
        if device_ids:
            ids = (ctypes.c_int64 * len(device_ids))(*device_ids)
            rc = lib.axon_start_nrt_profile(ids, len(device_ids))
        else:
            rc = lib.axon_start_nrt_profile(None, 0)
        if rc != 0:
            raise RuntimeError(f"axon_start_nrt_profile rc={rc}")
        try:
            yield
        finally:
            n = lib.axon_stop_nrt_profile(str(output_dir).encode())
            print(f"profile: {n} file(s) written to {output_dir}")

    mod.set_axon_ntff_profile_hook(_hook)


_install_axon_hooks()

B, S, D, H, HD = 2, 4096, 256, 4, 64
NCORES = 8
QR = S // NCORES          # 512 query rows per core
KC = S // 128             # 32 key chunks of 128
SCALE = 1.0 / np.sqrt(HD)

_BUILT = {}


def build_bass():
    if "nc" in _BUILT:
        return _BUILT["nc"]

    import concourse.tile as tile
    from concourse import bacc, mybir

    f32, bf16 = mybir.dt.float32, mybir.dt.bfloat16
    af = mybir.ActivationFunctionType

    nc = bacc.Bacc("TRN2", target_bir_lowering=False, debug=False, num_devices=NCORES)

    xt_in = nc.dram_tensor("xt", [B, 2, 128, S], bf16, kind="ExternalInput")
    xqt_in = nc.dram_tensor("xqt", [B, 2, 128, QR], bf16, kind="ExternalInput")
    g_in = nc.dram_tensor("g", [QR, S], f32, kind="ExternalInput")
    wq_in = nc.dram_tensor("wq", [2, 128, 256], bf16, kind="ExternalInput")
    wk_in = nc.dram_tensor("wk", [2, 128, 256], bf16, kind="ExternalInput")
    wv_in = nc.dram_tensor("wv", [2, 128, 260], bf16, kind="ExternalInput")
    sel_in = nc.dram_tensor("sel", [1, 260], bf16, kind="ExternalInput")
    wo_in = nc.dram_tensor("wo", [H, 64, 256], f32, kind="ExternalInput")
    bias_in = nc.dram_tensor("bias", [1, 256], f32, kind="ExternalInput")
    out_dram = nc.dram_tensor("out", [B, QR, 256], f32, kind="ExternalOutput")

    with tile.TileContext(nc) as tc, contextlib.ExitStack() as ctx:
        cp = ctx.enter_context(tc.tile_pool(name="const", bufs=1))
        # 3 slots x 2 banks for scores / QKV staging / out-proj, + 2 ctx accumulators
        ps_big = ctx.enter_context(tc.tile_pool(name="ps_big", bufs=3, space="PSUM"))
        ps_ct = ctx.enter_context(tc.tile_pool(name="ps_ct", bufs=1, space="PSUM"))

        # ---- constants / weights ----
        wq_sb = cp.tile([128, 2, 256], bf16, tag="wq")
        wk_sb = cp.tile([128, 2, 256], bf16, tag="wk")
        wv_sb = cp.tile([128, 2, 260], bf16, tag="wv")
        sel_sb = cp.tile([1, 260], bf16, tag="sel")
        bias_sb = cp.tile([1, 256], f32, tag="bias")
        ones_sb = cp.tile([1, 128], f32, tag="ones")
        ones_bf = cp.tile([1, 128], bf16, tag="ones_bf")
        for ic in range(2):
            nc.sync.dma_start(wq_sb[:, ic, :], wq_in[ic])
            nc.sync.dma_start(wk_sb[:, ic, :], wk_in[ic])
            nc.sync.dma_start(wv_sb[:, ic, :], wv_in[ic])
        nc.sync.dma_start(sel_sb[:], sel_in[:])
        nc.sync.dma_start(bias_sb[:], bias_in[:])
        nc.gpsimd.memset(ones_sb[:], 1.0)
        nc.gpsimd.memset(ones_bf[:], 1.0)
        wo_sb = []
        for h in range(H):
            t = cp.tile([64, 256], f32, tag=f"wo{h}", name=f"wo{h}")
            nc.sync.dma_start(t[:], wo_in[h])
            wo_sb.append(t)

        # ---- x^T (host-pretransposed, bf16) ----
        xt_sb = [[cp.tile([128, S], bf16, tag=f"xt{b}{ic}", name=f"xt{b}{ic}") for ic in range(2)] for b in range(B)]
        for b in range(B):
            for ic in range(2):
                nc.sync.dma_start(xt_sb[b][ic][:], xt_in[b, ic])

        # ---- all-(b,hp) qT upfront (frees the xqt staging before main pools) ----
        gt_sb = cp.tile([128, KC, QR], bf16, tag="gt")
        qts = [[cp.tile([128, QR], bf16, tag=f"qt{b}{hp}", name=f"qt{b}{hp}") for hp in range(2)] for b in range(B)]
        with tc.tile_pool(name="xqp", bufs=1) as xqp:
            xqt_sb = xqp.tile([128, B, 2, QR], bf16, tag="xqt")
            for b in range(B):
                for ic in range(2):
                    nc.sync.dma_start(xqt_sb[:, b, ic, :], xqt_in[b, ic])
            for b in range(B):
                for hp in range(2):
                    aux = ps_big.tile([128, 2, 512], f32, tag="sc", name="auxq")
                    for ic in range(2):
                        nc.tensor.matmul(
                            aux[:, 0, :QR], wq_sb[:, ic, hp * 128:(hp + 1) * 128],
                            xqt_sb[:, b, ic, :], start=(ic == 0), stop=(ic == 1),
                        )
                    nc.vector.tensor_copy(qts[b][hp][:], aux[:, 0, :QR])

        # ---- G^T build: load f32 rows, cast to bf16 (DVE), xbar-transpose.
        # All transposes go through nc.scalar's queue, kc-major so early key
        # chunks of gt become consumable while later ones are still in flight.
        with tc.tile_pool(name="graw", bufs=2) as pg, tc.tile_pool(name="gbf", bufs=1) as pgb:
            gbfs = []
            for qc in range(QR // 128):
                graw = pg.tile([128, S], f32, tag="graw")
                nc.sync.dma_start(graw[:], g_in[qc * 128:(qc + 1) * 128, :])
                gbf = pgb.tile([128, S], bf16, tag=f"gbf{qc}", name=f"gbf{qc}")
                nc.vector.tensor_copy(gbf[:], graw[:])
                gbfs.append(gbf)
            for kc in range(KC):
                for qc in range(QR // 128):
                    nc.scalar.dma_start_transpose(
                        gt_sb[:, kc, qc * 128:(qc + 1) * 128],
                        gbfs[qc][:, kc * 128:(kc + 1) * 128],
                    )

        ktp = ctx.enter_context(tc.tile_pool(name="ktp", bufs=2))
        vap = ctx.enter_context(tc.tile_pool(name="vap", bufs=2))
        ttp = ctx.enter_context(tc.tile_pool(name="ttp", bufs=2))
        pp = ctx.enter_context(tc.tile_pool(name="pp", bufs=3))
        otp = ctx.enter_context(tc.tile_pool(name="otp", bufs=2))

        den_all = cp.tile([1, 8, QR], f32, tag="den_all")
        ctf = [[None] * H for _ in range(B)]

        for b in range(B):
            for hp in range(2):
                qt = qts[b][hp]

                # ---- lazy K/V for this (batch, head-pair) ----
                kt = ktp.tile([128, S], bf16, tag="kt")
                for sc8 in range(8):
                    aux = ps_big.tile([128, 2, 512], f32, tag="sc", name="auxk")
                    for ic in range(2):
                        nc.tensor.matmul(
                            aux[:, 0, :], wk_sb[:, ic, hp * 128:(hp + 1) * 128],
                            xt_sb[b][ic][:, sc8 * 512:(sc8 + 1) * 512],
                            start=(ic == 0), stop=(ic == 1),
                        )
                    nc.scalar.copy(kt[:, sc8 * 512:(sc8 + 1) * 512], aux[:, 0, :])

                va = vap.tile([128, KC, 130], bf16, tag="va")
                for sg in range(KC // 2):
                    aux = ps_big.tile([128, 2, 512], f32, tag="sc", name="auxv")
                    for j in range(2):
                        kcj = sg * 2 + j
                        for ic in range(2):
                            nc.tensor.matmul(
                                aux[:, j, 0:130],
                                xt_sb[b][ic][:, kcj * 128:(kcj + 1) * 128],
                                wv_sb[:, ic, hp * 130:(hp + 1) * 130],
                                start=(ic == 0), stop=False,
                            )
                        nc.tensor.matmul(
                            aux[:, j, 0:130], ones_bf[0:1, :],
                            sel_sb[0:1, hp * 130:(hp + 1) * 130],
                            start=False, stop=True,
                        )
                    if sg % 2 == 0:
                        nc.vector.tensor_copy(va[:, sg * 2:(sg + 1) * 2, :], aux[:, 0:2, 0:130])
                    else:
                        nc.scalar.copy(va[:, sg * 2:(sg + 1) * 2, :], aux[:, 0:2, 0:130])

                # ---- main loop over key chunks ----
                ct0 = ps_ct.tile([65, QR], f32, tag="ct0")
                ct1 = ps_ct.tile([65, QR], f32, tag="ct1")
                for kc in range(KC):
                    scp = ps_big.tile([128, 2, QR], f32, tag="sc", name="scp")
                    nc.tensor.matmul(
                        scp[:, 0, :], kt[0:64, kc * 128:(kc + 1) * 128], qt[0:64, :],
                        start=True, stop=True, tile_position=(0, 0),
                    )
                    nc.tensor.matmul(
                        scp[:, 1, :], kt[64:128, kc * 128:(kc + 1) * 128], qt[64:128, :],
                        start=True, stop=True, tile_position=(64, 0),
                    )
                    tt = ttp.tile([128, 2, QR], f32, tag="tt")
                    nc.vector.tensor_mul(
                        tt[:, :, :], scp[:, :, :],
                        gt_sb[:, kc:kc + 1, :].broadcast_to([128, 2, QR]),
                    )
                    pt = pp.tile([128, 2, QR], bf16, tag="pt")
                    nc.scalar.activation(pt[:, :, :], tt[:, :, :], af.Exp)
                    nc.tensor.matmul(
                        ct0[:, :], va[:, kc, 0:65], pt[:, 0, :],
                        start=(kc == 0), stop=(kc == KC - 1),
                    )
                    nc.tensor.matmul(
                        ct1[:, :], va[:, kc, 65:130], pt[:, 1, :],
                        start=(kc == 0), stop=(kc == KC - 1),
                    )

                # ---- stash unnormalized ctx^T + denominators ----
                r = (b * 2 + hp) * 2
                c0 = cp.tile([64, QR], f32, tag=f"ctf{b}_{2 * hp}", name=f"ctf{b}_{2 * hp}")
                c1 = cp.tile([64, QR], f32, tag=f"ctf{b}_{2 * hp + 1}", name=f"ctf{b}_{2 * hp + 1}")
                nc.scalar.copy(c0[:], ct0[0:64, :])
                nc.scalar.copy(c1[:], ct1[0:64, :])
                nc.vector.tensor_copy(den_all[0:1, r, :], ct0[64:65, :])
                nc.vector.tensor_copy(den_all[0:1, r + 1, :], ct1[64:65, :])
                ctf[b][2 * hp] = c0
                ctf[b][2 * hp + 1] = c1

        # ---- reciprocal of all 8 denominator rows (partition-parallel) ----
        denp = cp.tile([128, 32], f32, tag="denp")
        nc.sync.dma_start(denp[:, :], den_all[0:1, :, :])
        recp = cp.tile([128, 32], f32, tag="recp")
        nc.vector.reciprocal(recp[:], denp[:])
        rec_all = cp.tile([1, 8, QR], f32, tag="rec_all")
        nc.sync.dma_start(rec_all[0:1, :, :], recp[:, :])

        # ---- scale ctx^T and project ----
        bcb = cp.tile([64, QR], f32, tag="bcb")
        for b in range(B):
            for h in range(H):
                r = b * 4 + h
                nc.sync.dma_start(bcb[:, :], rec_all[0:1, r, :].rearrange("p (o q) -> p o q", o=1).broadcast_to([1, 64, QR]))
                nc.vector.tensor_mul(ctf[b][h][:], ctf[b][h][:], bcb[:, :])
            for qs in range(QR // 128):
                op = ps_big.tile([128, 2, 512], f32, tag="sc", name="auxo")
                for h in range(H):
                    nc.tensor.matmul(
                        op[:, 0, 0:256], ctf[b][h][:, qs * 128:(qs + 1) * 128],
                        wo_sb[h][:], start=(h == 0), stop=False,
                    )
                nc.tensor.matmul(op[:, 0, 0:256], ones_sb[0:1, :], bias_sb[0:1, :], start=False, stop=True)
                ot = otp.tile([128, 256], f32, tag="ot")
                nc.vector.tensor_copy(ot[:], op[:, 0, 0:256])
                nc.sync.dma_start(out_dram[b, qs * 128:(qs + 1) * 128, :], ot[:])

    nc.compile()
    _BUILT["nc"] = nc
    return nc


def host_inputs(x, G, Wq, Wk, Wv, Wo, bo, b_extra):
    """Build the per-core input maps (layout prep + query-row sharding)."""
    import ml_dtypes

    f = np.float32
    bf = ml_dtypes.bfloat16
    x = np.asarray(x, f)
    G = np.asarray(G, f)
    xt = np.ascontiguousarray(x.transpose(0, 2, 1)).reshape(B, 2, 128, S).astype(bf)
    wq = np.ascontiguousarray(np.asarray(Wq, f).T * SCALE).reshape(2, 128, 256).astype(bf)
    wk = np.ascontiguousarray(np.asarray(Wk, f).T).reshape(2, 128, 256).astype(bf)
    wvt = np.asarray(Wv, f).T  # [256 in, 256 out]
    wv = np.zeros((2, 128, 260), f)
    for hp in range(2):
        wv[:, :, hp * 130:hp * 130 + 64] = wvt[:, hp * 128:hp * 128 + 64].reshape(2, 128, 64)
        wv[:, :, hp * 130 + 65:hp * 130 + 129] = wvt[:, hp * 128 + 64:hp * 128 + 128].reshape(2, 128, 64)
    wv = wv.astype(bf)
    sel = np.zeros((1, 260), f)
    sel[0, [64, 129, 194, 259]] = 1.0
    sel = sel.astype(bf)
    wo = np.ascontiguousarray(np.asarray(Wo, f).T).reshape(H, 64, 256)
    bias = (np.asarray(bo, f) + np.asarray(b_extra, f)).reshape(1, 256)

    shared = {"xt": xt, "wq": wq, "wk": wk, "wv": wv, "sel": sel, "wo": wo, "bias": bias}
    in_maps = []
    for c in range(NCORES):
        q0 = c * QR
        m = dict(shared)
        m["g"] = np.ascontiguousarray(G[q0:q0 + QR, :])
        m["xqt"] = np.ascontiguousarray(xt[:, :, :, q0:q0 + QR])
        in_maps.append(m)
    return in_maps


def run(in_maps, trace=False):
    from concourse.bass_utils import run_bass_kernel_spmd

    nc = build_bass()
    return run_bass_kernel_spmd(nc, in_maps, core_ids=list(range(NCORES)), trace=trace)


def kernel(x, G, Wq, Wk, Wv, Wo, bo, b_extra):
    in_maps = host_inputs(x, G, Wq, Wk, Wv, Wo, bo, b_extra)
    res = run(in_maps, trace=False)
    out = np.concatenate([res.results[c]["out"] for c in range(NCORES)], axis=1)
    return out.astype(np.float32)


# revision 11
# speedup vs baseline: 2.1626x; 1.2722x over previous
"""MultiHead HGNN attention (B=2, S=4096, D=256, H=4) on 8 TRN2 NeuronCores.

Sharding: query rows are split 8 ways (512 rows/core); every core computes all
batches/heads for its query block. The score matrix is built k-major
(scores^T) so the probs@V contraction needs no transposes; G^T is produced
on-device via a bf16 cast + xbar DMA transpose. Softmax denominators ride as
an extra ones-column in the V operand; normalization is applied to ctx^T via
a broadcast of 1/denom. Matmul operands are bf16 (fp32 streams at half rate
through the PE); accumulation stays fp32 in PSUM.
"""

import contextlib
import ctypes
import sys
import types

import numpy as np

sys.path.insert(0, "/opt/trn_rl_repo")


def _install_axon_hooks():
    """The agent image's antenv lacks axon_hooks; provide it so bass_utils can
    NTFF-profile under axon. Harmless when profiling is never requested."""
    if "antenv.axon_hooks" in sys.modules:
        return
    try:
        import antenv
    except ImportError:
        return
    mod = types.ModuleType("antenv.axon_hooks")
    holder = {}
    mod.set_axon_ntff_profile_hook = lambda h: holder.__setitem__("h", h)
    mod.get_axon_ntff_profile_hook = lambda: holder.get("h")
    sys.modules["antenv.axon_hooks"] = mod
    antenv.axon_hooks = mod
    try:
        lib = ctypes.CDLL("/opt/axon/libaxon_pjrt.so")
    except OSError:
        return
    if not hasattr(lib, "axon_start_nrt_profile"):
        return
    lib.axon_start_nrt_profile.argtypes = [ctypes.POINTER(ctypes.c_int64), ctypes.c_size_t]
    lib.axon_start_nrt_profile.restype = ctypes.c_int64
    lib.axon_stop_nrt_profile.argtypes = [ctypes.c_char_p]
    lib.axon_stop_nrt_profile.restype = ctypes.c_int64

    @contextlib.contextmanager
    def _hook(output_dir, device_ids):
        import jax

        jax.devices()
        if device_ids:
            ids = (ctypes.c_int64 * len(device_ids))(*device_ids)
            rc = lib.axon_start_nrt_profile(ids, len(device_ids))
        else:
            rc = lib.axon_start_nrt_profile(None, 0)
        if rc != 0:
            raise RuntimeError(f"axon_start_nrt_profile rc={rc}")
        try:
            yield
        finally:
            n = lib.axon_stop_nrt_profile(str(output_dir).encode())
            print(f"profile: {n} file(s) written to {output_dir}")

    mod.set_axon_ntff_profile_hook(_hook)


_install_axon_hooks()

B, S, D, H, HD = 2, 4096, 256, 4, 64
NCORES = 8
QR = S // NCORES          # 512 query rows per core
KC = S // 128             # 32 key chunks of 128
SCALE = 1.0 / np.sqrt(HD)

_BUILT = {}


def build_bass():
    if "nc" in _BUILT:
        return _BUILT["nc"]

    import concourse.tile as tile
    from concourse import bacc, mybir

    f32, bf16 = mybir.dt.float32, mybir.dt.bfloat16
    af = mybir.ActivationFunctionType

    nc = bacc.Bacc("TRN2", target_bir_lowering=False, debug=False, num_devices=NCORES)

    xt_in = nc.dram_tensor("xt", [B, 2, 128, S], bf16, kind="ExternalInput")
    xqt_in = nc.dram_tensor("xqt", [B, 2, 128, QR], bf16, kind="ExternalInput")
    g_in = nc.dram_tensor("g", [QR, S], f32, kind="ExternalInput")
    wq_in = nc.dram_tensor("wq", [2, 128, 256], bf16, kind="ExternalInput")
    wk_in = nc.dram_tensor("wk", [2, 128, 256], bf16, kind="ExternalInput")
    wv_in = nc.dram_tensor("wv", [2, 128, 260], bf16, kind="ExternalInput")
    sel_in = nc.dram_tensor("sel", [1, 260], bf16, kind="ExternalInput")
    wo_in = nc.dram_tensor("wo", [H, 64, 256], f32, kind="ExternalInput")
    bias_in = nc.dram_tensor("bias", [1, 256], f32, kind="ExternalInput")
    out_dram = nc.dram_tensor("out", [B, QR, 256], f32, kind="ExternalOutput")

    with tile.TileContext(nc) as tc, contextlib.ExitStack() as ctx:
        cp = ctx.enter_context(tc.tile_pool(name="const", bufs=1))
        # 3 slots x 2 banks for scores / QKV staging / out-proj, + 2 ctx accumulators
        ps_big = ctx.enter_context(tc.tile_pool(name="ps_big", bufs=3, space="PSUM"))
        ps_ct = ctx.enter_context(tc.tile_pool(name="ps_ct", bufs=1, space="PSUM"))

        # ---- constants / weights ----
        wq_sb = cp.tile([128, 2, 256], bf16, tag="wq")
        wk_sb = cp.tile([128, 2, 256], bf16, tag="wk")
        wv_sb = cp.tile([128, 2, 260], bf16, tag="wv")
        sel_sb = cp.tile([1, 260], bf16, tag="sel")
        bias_sb = cp.tile([1, 256], f32, tag="bias")
        ones_sb = cp.tile([1, 128], f32, tag="ones")
        ones_bf = cp.tile([1, 128], bf16, tag="ones_bf")
        for ic in range(2):
            nc.sync.dma_start(wq_sb[:, ic, :], wq_in[ic])
            nc.sync.dma_start(wk_sb[:, ic, :], wk_in[ic])
            nc.sync.dma_start(wv_sb[:, ic, :], wv_in[ic])
        nc.sync.dma_start(sel_sb[:], sel_in[:])
        nc.sync.dma_start(bias_sb[:], bias_in[:])
        nc.gpsimd.memset(ones_sb[:], 1.0)
        nc.gpsimd.memset(ones_bf[:], 1.0)
        wo_sb = []
        for h in range(H):
            t = cp.tile([64, 256], f32, tag=f"wo{h}", name=f"wo{h}")
            nc.sync.dma_start(t[:], wo_in[h])
            wo_sb.append(t)

        # ---- x^T (host-pretransposed, bf16) ----
        xt_sb = [[cp.tile([128, S], bf16, tag=f"xt{b}{ic}", name=f"xt{b}{ic}") for ic in range(2)] for b in range(B)]
        for b in range(B):
            for ic in range(2):
                nc.sync.dma_start(xt_sb[b][ic][:], xt_in[b, ic])

        # ---- all-(b,hp) qT upfront (frees the xqt staging before main pools) ----
        gt_sb = cp.tile([128, KC, QR], bf16, tag="gt")
        qts = [[cp.tile([128, QR], bf16, tag=f"qt{b}{hp}", name=f"qt{b}{hp}") for hp in range(2)] for b in range(B)]
        with tc.tile_pool(name="xqp", bufs=1) as xqp:
            xqt_sb = xqp.tile([128, B, 2, QR], bf16, tag="xqt")
            for b in range(B):
                for ic in range(2):
                    nc.sync.dma_start(xqt_sb[:, b, ic, :], xqt_in[b, ic])
            for b in range(B):
                for hp in range(2):
                    aux = ps_big.tile([128, 2, 512], f32, tag="sc", name="auxq")
                    for ic in range(2):
                        nc.tensor.matmul(
                            aux[:, 0, :QR], wq_sb[:, ic, hp * 128:(hp + 1) * 128],
                            xqt_sb[:, b, ic, :], start=(ic == 0), stop=(ic == 1),
                        )
                    nc.vector.tensor_copy(qts[b][hp][:], aux[:, 0, :QR])

        # ---- G^T build: load f32 rows, cast to bf16 (DVE), xbar-transpose.
        # All transposes go through nc.scalar's queue, kc-major so early key
        # chunks of gt become consumable while later ones are still in flight.
        with tc.tile_pool(name="graw", bufs=2) as pg, tc.tile_pool(name="gbf", bufs=1) as pgb:
            gbfs = []
            for qc in range(QR // 128):
                graw = pg.tile([128, S], f32, tag="graw")
                nc.sync.dma_start(graw[:], g_in[qc * 128:(qc + 1) * 128, :])
                gbf = pgb.tile([128, S], bf16, tag=f"gbf{qc}", name=f"gbf{qc}")
                nc.vector.tensor_copy(gbf[:], graw[:])
                gbfs.append(gbf)
            for kp in range(8):
                for qc in range(QR // 128):
                    nc.sync.dma_start_transpose(
                        gt_sb[:, kp * 4:(kp + 1) * 4, qc * 128:(qc + 1) * 128],
                        gbfs[qc][:, kp * 512:(kp + 1) * 512],
                    )

        ktp = ctx.enter_context(tc.tile_pool(name="ktp", bufs=2))
        vap = ctx.enter_context(tc.tile_pool(name="vap", bufs=2))
        ttp = ctx.enter_context(tc.tile_pool(name="ttp", bufs=2))
        pp = ctx.enter_context(tc.tile_pool(name="pp", bufs=3))
        otp = ctx.enter_context(tc.tile_pool(name="otp", bufs=2))

        den_all = cp.tile([1, 8, QR], f32, tag="den_all")
        ctf = [[None] * H for _ in range(B)]

        for b in range(B):
            for hp in range(2):
                qt = qts[b][hp]

                # ---- lazy K/V for this (batch, head-pair) ----
                kt = ktp.tile([128, S], bf16, tag="kt")
                for sc8 in range(8):
                    aux = ps_big.tile([128, 2, 512], f32, tag="sc", name="auxk")
                    for ic in range(2):
                        nc.tensor.matmul(
                            aux[:, 0, :], wk_sb[:, ic, hp * 128:(hp + 1) * 128],
                            xt_sb[b][ic][:, sc8 * 512:(sc8 + 1) * 512],
                            start=(ic == 0), stop=(ic == 1),
                        )
                    nc.scalar.copy(kt[:, sc8 * 512:(sc8 + 1) * 512], aux[:, 0, :])

                va = vap.tile([128, KC, 130], bf16, tag="va")
                for sg in range(KC // 2):
                    aux = ps_big.tile([128, 2, 512], f32, tag="sc", name="auxv")
                    for j in range(2):
                        kcj = sg * 2 + j
                        for ic in range(2):
                            nc.tensor.matmul(
                                aux[:, j, 0:130],
                                xt_sb[b][ic][:, kcj * 128:(kcj + 1) * 128],
                                wv_sb[:, ic, hp * 130:(hp + 1) * 130],
                                start=(ic == 0), stop=False,
                            )
                        nc.tensor.matmul(
                            aux[:, j, 0:130], ones_bf[0:1, :],
                            sel_sb[0:1, hp * 130:(hp + 1) * 130],
                            start=False, stop=True,
                        )
                    nc.scalar.copy(va[:, sg * 2:(sg + 1) * 2, :], aux[:, 0:2, 0:130])

                # ---- main loop over key chunks ----
                ct0 = ps_ct.tile([65, QR], f32, tag="ct0")
                ct1 = ps_ct.tile([65, QR], f32, tag="ct1")
                for kc in range(KC):
                    scp = ps_big.tile([128, 2, QR], f32, tag="sc", name="scp")
                    nc.tensor.matmul(
                        scp[:, 0, :], kt[0:64, kc * 128:(kc + 1) * 128], qt[0:64, :],
                        start=True, stop=True, tile_position=(0, 0),
                    )
                    nc.tensor.matmul(
                        scp[:, 1, :], kt[64:128, kc * 128:(kc + 1) * 128], qt[64:128, :],
                        start=True, stop=True, tile_position=(64, 0),
                    )
                    tt = ttp.tile([128, 2 * QR], f32, tag="tt")
                    nc.vector.tensor_mul(tt[:, 0:QR], scp[:, 0, :], gt_sb[:, kc, :])
                    nc.vector.tensor_mul(tt[:, QR:2 * QR], scp[:, 1, :], gt_sb[:, kc, :])
                    pt = pp.tile([128, 2, QR], bf16, tag="pt")
                    nc.scalar.activation(pt[:, :, :].rearrange("p a b -> p (a b)"), tt[:, :], af.Exp)
                    nc.tensor.matmul(
                        ct0[:, :], va[:, kc, 0:65], pt[:, 0, :],
                        start=(kc == 0), stop=(kc == KC - 1),
                    )
                    nc.tensor.matmul(
                        ct1[:, :], va[:, kc, 65:130], pt[:, 1, :],
                        start=(kc == 0), stop=(kc == KC - 1),
                    )

                # ---- stash unnormalized ctx^T + denominators ----
                r = (b * 2 + hp) * 2
                c0 = cp.tile([64, QR], f32, tag=f"ctf{b}_{2 * hp}", name=f"ctf{b}_{2 * hp}")
                c1 = cp.tile([64, QR], f32, tag=f"ctf{b}_{2 * hp + 1}", name=f"ctf{b}_{2 * hp + 1}")
                nc.scalar.copy(c0[:], ct0[0:64, :])
                nc.scalar.copy(c1[:], ct1[0:64, :])
                nc.vector.tensor_copy(den_all[0:1, r, :], ct0[64:65, :])
                nc.vector.tensor_copy(den_all[0:1, r + 1, :], ct1[64:65, :])
                ctf[b][2 * hp] = c0
                ctf[b][2 * hp + 1] = c1

        # ---- reciprocal of all 8 denominator rows (partition-parallel) ----
        denp = cp.tile([128, 32], f32, tag="denp")
        nc.sync.dma_start(denp[:, :], den_all[0:1, :, :])
        recp = cp.tile([128, 32], f32, tag="recp")
        nc.vector.reciprocal(recp[:], denp[:])
        rec_all = cp.tile([1, 8, QR], f32, tag="rec_all")
        nc.sync.dma_start(rec_all[0:1, :, :], recp[:, :])

        # ---- scale ctx^T and project ----
        bcb = cp.tile([64, QR], f32, tag="bcb")
        for b in range(B):
            for h in range(H):
                r = b * 4 + h
                nc.sync.dma_start(bcb[:, :], rec_all[0:1, r, :].rearrange("p (o q) -> p o q", o=1).broadcast_to([1, 64, QR]))
                nc.vector.tensor_mul(ctf[b][h][:], ctf[b][h][:], bcb[:, :])
            for qs in range(QR // 128):
                op = ps_big.tile([128, 2, 512], f32, tag="sc", name="auxo")
                for h in range(H):
                    nc.tensor.matmul(
                        op[:, 0, 0:256], ctf[b][h][:, qs * 128:(qs + 1) * 128],
                        wo_sb[h][:], start=(h == 0), stop=False,
                    )
                nc.tensor.matmul(op[:, 0, 0:256], ones_sb[0:1, :], bias_sb[0:1, :], start=False, stop=True)
                ot = otp.tile([128, 256], f32, tag="ot")
                nc.vector.tensor_copy(ot[:], op[:, 0, 0:256])
                nc.sync.dma_start(out_dram[b, qs * 128:(qs + 1) * 128, :], ot[:])

    nc.compile()
    _BUILT["nc"] = nc
    return nc


def host_inputs(x, G, Wq, Wk, Wv, Wo, bo, b_extra):
    """Build the per-core input maps (layout prep + query-row sharding)."""
    import ml_dtypes

    f = np.float32
    bf = ml_dtypes.bfloat16
    x = np.asarray(x, f)
    G = np.asarray(G, f)
    xt = np.ascontiguousarray(x.transpose(0, 2, 1)).reshape(B, 2, 128, S).astype(bf)
    wq = np.ascontiguousarray(np.asarray(Wq, f).T * SCALE).reshape(2, 128, 256).astype(bf)
    wk = np.ascontiguousarray(np.asarray(Wk, f).T).reshape(2, 128, 256).astype(bf)
    wvt = np.asarray(Wv, f).T  # [256 in, 256 out]
    wv = np.zeros((2, 128, 260), f)
    for hp in range(2):
        wv[:, :, hp * 130:hp * 130 + 64] = wvt[:, hp * 128:hp * 128 + 64].reshape(2, 128, 64)
        wv[:, :, hp * 130 + 65:hp * 130 + 129] = wvt[:, hp * 128 + 64:hp * 128 + 128].reshape(2, 128, 64)
    wv = wv.astype(bf)
    sel = np.zeros((1, 260), f)
    sel[0, [64, 129, 194, 259]] = 1.0
    sel = sel.astype(bf)
    wo = np.ascontiguousarray(np.asarray(Wo, f).T).reshape(H, 64, 256)
    bias = (np.asarray(bo, f) + np.asarray(b_extra, f)).reshape(1, 256)

    shared = {"xt": xt, "wq": wq, "wk": wk, "wv": wv, "sel": sel, "wo": wo, "bias": bias}
    in_maps = []
    for c in range(NCORES):
        q0 = c * QR
        m = dict(shared)
        m["g"] = np.ascontiguousarray(G[q0:q0 + QR, :])
        m["xqt"] = np.ascontiguousarray(xt[:, :, :, q0:q0 + QR])
        in_maps.append(m)
    return in_maps


def run(in_maps, trace=False):
    from concourse.bass_utils import run_bass_kernel_spmd

    nc = build_bass()
    return run_bass_kernel_spmd(nc, in_maps, core_ids=list(range(NCORES)), trace=trace)


def kernel(x, G, Wq, Wk, Wv, Wo, bo, b_extra):
    in_maps = host_inputs(x, G, Wq, Wk, Wv, Wo, bo, b_extra)
    res = run(in_maps, trace=False)
    out = np.concatenate([res.results[c]["out"] for c in range(NCORES)], axis=1)
    return out.astype(np.float32)


# revision 12
# speedup vs baseline: 2.3667x; 1.0944x over previous
"""MultiHead HGNN attention (B=2, S=4096, D=256, H=4) on 8 TRN2 NeuronCores.

Sharding: query rows are split 8 ways (512 rows/core); every core computes all
batches/heads for its query block. The score matrix is built k-major
(scores^T) so the probs@V contraction needs no transposes; G^T is produced
on-device via a bf16 cast + xbar DMA transpose. Softmax denominators ride as
an extra ones-column in the V operand; normalization is applied to ctx^T via
a broadcast of 1/denom. Matmul operands are bf16 (fp32 streams at half rate
through the PE); accumulation stays fp32 in PSUM.
"""

import contextlib
import ctypes
import sys
import types

import numpy as np

sys.path.insert(0, "/opt/trn_rl_repo")


def _install_axon_hooks():
    """The agent image's antenv lacks axon_hooks; provide it so bass_utils can
    NTFF-profile under axon. Harmless when profiling is never requested."""
    if "antenv.axon_hooks" in sys.modules:
        return
    try:
        import antenv
    except ImportError:
        return
    mod = types.ModuleType("antenv.axon_hooks")
    holder = {}
    mod.set_axon_ntff_profile_hook = lambda h: holder.__setitem__("h", h)
    mod.get_axon_ntff_profile_hook = lambda: holder.get("h")
    sys.modules["antenv.axon_hooks"] = mod
    antenv.axon_hooks = mod
    try:
        lib = ctypes.CDLL("/opt/axon/libaxon_pjrt.so")
    except OSError:
        return
    if not hasattr(lib, "axon_start_nrt_profile"):
        return
    lib.axon_start_nrt_profile.argtypes = [ctypes.POINTER(ctypes.c_int64), ctypes.c_size_t]
    lib.axon_start_nrt_profile.restype = ctypes.c_int64
    lib.axon_stop_nrt_profile.argtypes = [ctypes.c_char_p]
    lib.axon_stop_nrt_profile.restype = ctypes.c_int64

    @contextlib.contextmanager
    def _hook(output_dir, device_ids):
        import jax

        jax.devices()
        if device_ids:
            ids = (ctypes.c_int64 * len(device_ids))(*device_ids)
            rc = lib.axon_start_nrt_profile(ids, len(device_ids))
        else:
            rc = lib.axon_start_nrt_profile(None, 0)
        if rc != 0:
            raise RuntimeError(f"axon_start_nrt_profile rc={rc}")
        try:
            yield
        finally:
            n = lib.axon_stop_nrt_profile(str(output_dir).encode())
            print(f"profile: {n} file(s) written to {output_dir}")

    mod.set_axon_ntff_profile_hook(_hook)


_install_axon_hooks()

B, S, D, H, HD = 2, 4096, 256, 4, 64
NCORES = 8
QR = S // NCORES          # 512 query rows per core
KC = S // 128             # 32 key chunks of 128
SCALE = 1.0 / np.sqrt(HD)

_BUILT = {}


def build_bass():
    if "nc" in _BUILT:
        return _BUILT["nc"]

    import concourse.tile as tile
    from concourse import bacc, mybir

    f32, bf16 = mybir.dt.float32, mybir.dt.bfloat16
    af = mybir.ActivationFunctionType

    nc = bacc.Bacc("TRN2", target_bir_lowering=False, debug=False, num_devices=NCORES)

    xt_in = nc.dram_tensor("xt", [B, 2, 128, S], bf16, kind="ExternalInput")
    xqt_in = nc.dram_tensor("xqt", [B, 2, 128, QR], bf16, kind="ExternalInput")
    g_in = nc.dram_tensor("g", [QR, S], f32, kind="ExternalInput")
    wq_in = nc.dram_tensor("wq", [2, 128, 256], bf16, kind="ExternalInput")
    wk_in = nc.dram_tensor("wk", [2, 128, 256], bf16, kind="ExternalInput")
    wv_in = nc.dram_tensor("wv", [2, 128, 260], bf16, kind="ExternalInput")
    sel_in = nc.dram_tensor("sel", [1, 260], bf16, kind="ExternalInput")
    wo_in = nc.dram_tensor("wo", [H, 64, 256], f32, kind="ExternalInput")
    bias_in = nc.dram_tensor("bias", [1, 256], f32, kind="ExternalInput")
    out_dram = nc.dram_tensor("out", [B, QR, 256], f32, kind="ExternalOutput")

    with tile.TileContext(nc) as tc, contextlib.ExitStack() as ctx:
        cp = ctx.enter_context(tc.tile_pool(name="const", bufs=1))
        # 3 slots x 2 banks for scores / QKV staging / out-proj, + 2 ctx accumulators
        ps_big = ctx.enter_context(tc.tile_pool(name="ps_big", bufs=3, space="PSUM"))
        ps_ct = ctx.enter_context(tc.tile_pool(name="ps_ct", bufs=1, space="PSUM"))

        # ---- constants / weights ----
        wq_sb = cp.tile([128, 2, 256], bf16, tag="wq")
        wk_sb = cp.tile([128, 2, 256], bf16, tag="wk")
        wv_sb = cp.tile([128, 2, 260], bf16, tag="wv")
        sel_sb = cp.tile([1, 260], bf16, tag="sel")
        bias_sb = cp.tile([1, 256], f32, tag="bias")
        ones_sb = cp.tile([1, 128], f32, tag="ones")
        ones_bf = cp.tile([1, 128], bf16, tag="ones_bf")
        for ic in range(2):
            nc.sync.dma_start(wq_sb[:, ic, :], wq_in[ic])
            nc.sync.dma_start(wk_sb[:, ic, :], wk_in[ic])
            nc.sync.dma_start(wv_sb[:, ic, :], wv_in[ic])
        nc.sync.dma_start(sel_sb[:], sel_in[:])
        nc.sync.dma_start(bias_sb[:], bias_in[:])
        nc.gpsimd.memset(ones_sb[:], 1.0)
        nc.gpsimd.memset(ones_bf[:], 1.0)
        wo_sb = []
        for h in range(H):
            t = cp.tile([64, 256], f32, tag=f"wo{h}", name=f"wo{h}")
            nc.sync.dma_start(t[:], wo_in[h])
            wo_sb.append(t)

        # ---- x^T (host-pretransposed, bf16) ----
        xt_sb = [[cp.tile([128, S], bf16, tag=f"xt{b}{ic}", name=f"xt{b}{ic}") for ic in range(2)] for b in range(B)]
        for b in range(B):
            for ic in range(2):
                nc.sync.dma_start(xt_sb[b][ic][:], xt_in[b, ic])

        # ---- all-(b,hp) qT upfront (frees the xqt staging before main pools) ----
        gt_sb = cp.tile([128, KC, QR], bf16, tag="gt")
        qts = [[cp.tile([128, QR], bf16, tag=f"qt{b}{hp}", name=f"qt{b}{hp}") for hp in range(2)] for b in range(B)]
        with tc.tile_pool(name="xqp", bufs=1) as xqp:
            xqt_sb = xqp.tile([128, B, 2, QR], bf16, tag="xqt")
            for b in range(B):
                for ic in range(2):
                    nc.sync.dma_start(xqt_sb[:, b, ic, :], xqt_in[b, ic])
            for b in range(B):
                for hp in range(2):
                    aux = ps_big.tile([128, 2, 512], f32, tag="sc", name="auxq")
                    for ic in range(2):
                        nc.tensor.matmul(
                            aux[:, 0, :QR], wq_sb[:, ic, hp * 128:(hp + 1) * 128],
                            xqt_sb[:, b, ic, :], start=(ic == 0), stop=(ic == 1),
                        )
                    nc.vector.tensor_copy(qts[b][hp][:], aux[:, 0, :QR])

        # ---- G^T build: load f32 rows, cast to bf16 (DVE), xbar-transpose.
        # All transposes go through nc.scalar's queue, kc-major so early key
        # chunks of gt become consumable while later ones are still in flight.
        with tc.tile_pool(name="graw", bufs=2) as pg, tc.tile_pool(name="gbf", bufs=1) as pgb:
            gbfs = []
            for qc in range(QR // 128):
                graw = pg.tile([128, S], f32, tag="graw")
                nc.sync.dma_start(graw[:], g_in[qc * 128:(qc + 1) * 128, :])
                gbf = pgb.tile([128, S], bf16, tag=f"gbf{qc}", name=f"gbf{qc}")
                nc.vector.tensor_copy(gbf[:], graw[:])
                gbfs.append(gbf)
            for kp in range(8):
                for qc in range(QR // 128):
                    nc.sync.dma_start_transpose(
                        gt_sb[:, kp * 4:(kp + 1) * 4, qc * 128:(qc + 1) * 128],
                        gbfs[qc][:, kp * 512:(kp + 1) * 512],
                    )

        ktp = ctx.enter_context(tc.tile_pool(name="ktp", bufs=2))
        vap = ctx.enter_context(tc.tile_pool(name="vap", bufs=2))
        ttp = ctx.enter_context(tc.tile_pool(name="ttp", bufs=2))
        pp = ctx.enter_context(tc.tile_pool(name="pp", bufs=3))
        otp = ctx.enter_context(tc.tile_pool(name="otp", bufs=2))

        den_all = cp.tile([1, 8, QR], f32, tag="den_all")
        ctf = [[None] * H for _ in range(B)]

        for b in range(B):
            for hp in range(2):
                qt = qts[b][hp]

                # ---- lazy K/V for this (batch, head-pair) ----
                kt = ktp.tile([128, S], bf16, tag="kt")
                for sc8 in range(8):
                    aux = ps_big.tile([128, 2, 512], f32, tag="sc", name="auxk")
                    for ic in range(2):
                        nc.tensor.matmul(
                            aux[:, 0, :], wk_sb[:, ic, hp * 128:(hp + 1) * 128],
                            xt_sb[b][ic][:, sc8 * 512:(sc8 + 1) * 512],
                            start=(ic == 0), stop=(ic == 1),
                        )
                    nc.scalar.copy(kt[:, sc8 * 512:(sc8 + 1) * 512], aux[:, 0, :])

                va = vap.tile([128, KC, 130], bf16, tag="va")
                for sg in range(KC // 2):
                    aux = ps_big.tile([128, 2, 512], f32, tag="sc", name="auxv")
                    for j in range(2):
                        kcj = sg * 2 + j
                        for ic in range(2):
                            nc.tensor.matmul(
                                aux[:, j, 0:130],
                                xt_sb[b][ic][:, kcj * 128:(kcj + 1) * 128],
                                wv_sb[:, ic, hp * 130:(hp + 1) * 130],
                                start=(ic == 0), stop=(ic == 1),
                            )
                    nc.scalar.copy(va[:, sg * 2:(sg + 1) * 2, :], aux[:, 0:2, 0:130])
                nc.gpsimd.memset(va[:, :, 64:65], 1.0)
                nc.gpsimd.memset(va[:, :, 129:130], 1.0)

                # ---- main loop over key chunks ----
                ct0 = ps_ct.tile([65, QR], f32, tag="ct0")
                ct1 = ps_ct.tile([65, QR], f32, tag="ct1")
                for kc in range(KC):
                    scp = ps_big.tile([128, 2, QR], f32, tag="sc", name="scp")
                    nc.tensor.matmul(
                        scp[:, 0, :], kt[0:64, kc * 128:(kc + 1) * 128], qt[0:64, :],
                        start=True, stop=True, tile_position=(0, 0),
                    )
                    nc.tensor.matmul(
                        scp[:, 1, :], kt[64:128, kc * 128:(kc + 1) * 128], qt[64:128, :],
                        start=True, stop=True, tile_position=(64, 0),
                    )
                    tt = ttp.tile([128, 2 * QR], f32, tag="tt")
                    nc.vector.tensor_mul(tt[:, 0:QR], scp[:, 0, :], gt_sb[:, kc, :])
                    nc.vector.tensor_mul(tt[:, QR:2 * QR], scp[:, 1, :], gt_sb[:, kc, :])
                    pt = pp.tile([128, 2, QR], bf16, tag="pt")
                    nc.scalar.activation(pt[:, :, :].rearrange("p a b -> p (a b)"), tt[:, :], af.Exp)
                    nc.tensor.matmul(
                        ct0[:, :], va[:, kc, 0:65], pt[:, 0, :],
                        start=(kc == 0), stop=(kc == KC - 1),
                    )
                    nc.tensor.matmul(
                        ct1[:, :], va[:, kc, 65:130], pt[:, 1, :],
                        start=(kc == 0), stop=(kc == KC - 1),
                    )

                # ---- stash unnormalized ctx^T + denominators ----
                r = (b * 2 + hp) * 2
                c0 = cp.tile([64, QR], f32, tag=f"ctf{b}_{2 * hp}", name=f"ctf{b}_{2 * hp}")
                c1 = cp.tile([64, QR], f32, tag=f"ctf{b}_{2 * hp + 1}", name=f"ctf{b}_{2 * hp + 1}")
                nc.scalar.copy(c0[:], ct0[0:64, :])
                nc.scalar.copy(c1[:], ct1[0:64, :])
                nc.vector.tensor_copy(den_all[0:1, r, :], ct0[64:65, :])
                nc.vector.tensor_copy(den_all[0:1, r + 1, :], ct1[64:65, :])
                ctf[b][2 * hp] = c0
                ctf[b][2 * hp + 1] = c1

        # ---- reciprocal of all 8 denominator rows (partition-parallel) ----
        denp = cp.tile([128, 32], f32, tag="denp")
        nc.sync.dma_start(denp[:, :], den_all[0:1, :, :])
        recp = cp.tile([128, 32], f32, tag="recp")
        nc.vector.reciprocal(recp[:], denp[:])
        rec_all = cp.tile([1, 8, QR], f32, tag="rec_all")
        nc.sync.dma_start(rec_all[0:1, :, :], recp[:, :])

        # ---- scale ctx^T and project ----
        bcb = cp.tile([64, QR], f32, tag="bcb")
        for b in range(B):
            for h in range(H):
                r = b * 4 + h
                nc.sync.dma_start(bcb[:, :], rec_all[0:1, r, :].rearrange("p (o q) -> p o q", o=1).broadcast_to([1, 64, QR]))
                nc.vector.tensor_mul(ctf[b][h][:], ctf[b][h][:], bcb[:, :])
            for qs in range(QR // 128):
                op = ps_big.tile([128, 2, 512], f32, tag="sc", name="auxo")
                for h in range(H):
                    nc.tensor.matmul(
                        op[:, 0, 0:256], ctf[b][h][:, qs * 128:(qs + 1) * 128],
                        wo_sb[h][:], start=(h == 0), stop=False,
                    )
                nc.tensor.matmul(op[:, 0, 0:256], ones_sb[0:1, :], bias_sb[0:1, :], start=False, stop=True)
                ot = otp.tile([128, 256], f32, tag="ot")
                nc.vector.tensor_copy(ot[:], op[:, 0, 0:256])
                nc.sync.dma_start(out_dram[b, qs * 128:(qs + 1) * 128, :], ot[:])

    nc.compile()
    _BUILT["nc"] = nc
    return nc


def host_inputs(x, G, Wq, Wk, Wv, Wo, bo, b_extra):
    """Build the per-core input maps (layout prep + query-row sharding)."""
    import ml_dtypes

    f = np.float32
    bf = ml_dtypes.bfloat16
    x = np.asarray(x, f)
    G = np.asarray(G, f)
    xt = np.ascontiguousarray(x.transpose(0, 2, 1)).reshape(B, 2, 128, S).astype(bf)
    wq = np.ascontiguousarray(np.asarray(Wq, f).T * SCALE).reshape(2, 128, 256).astype(bf)
    wk = np.ascontiguousarray(np.asarray(Wk, f).T).reshape(2, 128, 256).astype(bf)
    wvt = np.asarray(Wv, f).T  # [256 in, 256 out]
    wv = np.zeros((2, 128, 260), f)
    for hp in range(2):
        wv[:, :, hp * 130:hp * 130 + 64] = wvt[:, hp * 128:hp * 128 + 64].reshape(2, 128, 64)
        wv[:, :, hp * 130 + 65:hp * 130 + 129] = wvt[:, hp * 128 + 64:hp * 128 + 128].reshape(2, 128, 64)
    wv = wv.astype(bf)
    sel = np.zeros((1, 260), f)
    sel[0, [64, 129, 194, 259]] = 1.0
    sel = sel.astype(bf)
    wo = np.ascontiguousarray(np.asarray(Wo, f).T).reshape(H, 64, 256)
    bias = (np.asarray(bo, f) + np.asarray(b_extra, f)).reshape(1, 256)

    shared = {"xt": xt, "wq": wq, "wk": wk, "wv": wv, "sel": sel, "wo": wo, "bias": bias}
    in_maps = []
    for c in range(NCORES):
        q0 = c * QR
        m = dict(shared)
        m["g"] = np.ascontiguousarray(G[q0:q0 + QR, :])
        m["xqt"] = np.ascontiguousarray(xt[:, :, :, q0:q0 + QR])
        in_maps.append(m)
    return in_maps


def run(in_maps, trace=False):
    from concourse.bass_utils import run_bass_kernel_spmd

    nc = build_bass()
    return run_bass_kernel_spmd(nc, in_maps, core_ids=list(range(NCORES)), trace=trace)


def kernel(x, G, Wq, Wk, Wv, Wo, bo, b_extra):
    in_maps = host_inputs(x, G, Wq, Wk, Wv, Wo, bo, b_extra)
    res = run(in_maps, trace=False)
    out = np.concatenate([res.results[c]["out"] for c in range(NCORES)], axis=1)
    return out.astype(np.float32)
